# revision 5
# baseline (speedup 1.0000x reference)
"""Trainium2 Bass kernel for nn_CADense (context-adaptive low-rank dense layer).

Computes, for the full batch:
    s_mod = s + context @ w          # [B, R]
    low   = (data @ u) * s_mod       # [B, R]
    out   = relu(low @ v.T + 2*bias) # [B, UNITS]

Sharding: data-parallel over batch across 8 NeuronCores; u/s/v/w/bias
replicated. Each core runs the same Bass program on its 1024-row shard.
"""

import os
import sys
from contextlib import ExitStack

import numpy as np


def _ensure_concourse():
    try:
        import concourse  # noqa: F401
    except ImportError:
        for p in ("/opt/trn_rl_repo", "/root/.axon_site/_ro/trn_rl_repo"):
            if os.path.isdir(p) and p not in sys.path:
                sys.path.insert(0, p)


_ensure_concourse()

import concourse.tile as tile  # noqa: E402
from concourse import bacc, mybir  # noqa: E402
from concourse.bass_utils import run_bass_kernel_spmd  # noqa: E402
from concourse.masks import make_identity  # noqa: E402

NCORES = 8
B, N_IN, UNITS, RANK, CCTX = 8192, 2048, 2048, 256, 512
NB = B // NCORES  # batch rows per core
P = 128
BT = 512  # batch tile (free dim of T-domain matmuls)
NBT = NB // BT  # batch tiles per core
KC = N_IN // P  # 16 contraction chunks for data @ u
CC = CCTX // P  # 4 contraction chunks for context @ w
RC = RANK // P  # 2 rank chunks
MS = 512  # output units slice width
NMS = UNITS // MS  # 4 unit slices

F32 = mybir.dt.float32
F32R = mybir.dt.float32r


def _emit(nc, tc, ctx):
    d_data = nc.dram_tensor("data", [NB, N_IN], F32R, kind="ExternalInput")
    d_ctx = nc.dram_tensor("context", [NB, CCTX], F32R, kind="ExternalInput")
    d_u = nc.dram_tensor("u", [N_IN, RANK], F32R, kind="ExternalInput")
    d_s = nc.dram_tensor("s", [RANK], F32, kind="ExternalInput")
    d_v = nc.dram_tensor("v", [UNITS, RANK], F32R, kind="ExternalInput")
    d_w = nc.dram_tensor("w", [CCTX, RANK], F32R, kind="ExternalInput")
    d_bias = nc.dram_tensor("bias", [UNITS], F32R, kind="ExternalInput")
    d_out = nc.dram_tensor("out", [NB, UNITS], F32, kind="ExternalOutput")

    singles = ctx.enter_context(tc.tile_pool(name="singles", bufs=1))
    tp_psum = ctx.enter_context(tc.tile_pool(name="tp_psum", bufs=2, space="PSUM"))
    du_psum = ctx.enter_context(tc.tile_pool(name="du_psum", bufs=2, space="PSUM"))
    s_psum = ctx.enter_context(tc.tile_pool(name="s_psum", bufs=2, space="PSUM"))
    o_psum = ctx.enter_context(tc.tile_pool(name="o_psum", bufs=2, space="PSUM"))
    dpool = ctx.enter_context(tc.tile_pool(name="data_nat", bufs=3))
    dTpool = ctx.enter_context(tc.tile_pool(name="dataT", bufs=2))
    cpool = ctx.enter_context(tc.tile_pool(name="ctx_nat", bufs=3))
    cTpool = ctx.enter_context(tc.tile_pool(name="ctxT", bufs=2))
    lowpool = ctx.enter_context(tc.tile_pool(name="lowT", bufs=2))
    smodpool = ctx.enter_context(tc.tile_pool(name="smod", bufs=2))
    opool = ctx.enter_context(tc.tile_pool(name="outsb", bufs=4))

    identity_f = singles.tile([P, P], F32)
    make_identity(nc, identity_f[:])
    identity = singles.tile([P, P], F32R)
    nc.vector.tensor_copy(out=identity[:], in_=identity_f[:])
    ident_r = identity[:]

    # Weights, replicated per core, loaded once.
    u_sb = singles.tile([P, KC, RANK], F32R)
    nc.sync.dma_start(out=u_sb[:], in_=d_u.ap().rearrange("(kc p) r -> p kc r", p=P))
    w_sb = singles.tile([P, CC, RANK], F32R)
    nc.sync.dma_start(out=w_sb[:], in_=d_w.ap().rearrange("(cc p) r -> p cc r", p=P))
    s_sb = singles.tile([P, RC], F32)
    nc.sync.dma_start(out=s_sb[:], in_=d_s.ap().rearrange("(rc p) -> p rc", p=P))
    bias2 = singles.tile([1, UNITS], F32R)
    nc.sync.dma_start(out=bias2[:], in_=d_bias.ap().rearrange("(a m) -> a m", a=1))
    ones_f = singles.tile([1, P], F32)
    nc.vector.memset(ones_f[:], 2.0)
    ones = singles.tile([1, P], F32R)
    nc.vector.tensor_copy(out=ones[:], in_=ones_f[:])

    # vT[r, m] built on-chip from v[m, r] via PE transpose.
    vT_sb = singles.tile([P, RC, UNITS], F32R)
    with tc.tile_pool(name="vload", bufs=1) as vpool:
        v_nat = vpool.tile([P, UNITS // P, RANK], F32R)
        nc.sync.dma_start(
            out=v_nat[:], in_=d_v.ap().rearrange("(mc p) r -> p mc r", p=P)
        )
        for mc in range(UNITS // P):
            for rc in range(RC):
                pt = tp_psum.tile([P, P], F32R)
                nc.tensor.transpose(
                    pt[:], v_nat[:, mc, rc * P : (rc + 1) * P], ident_r
                )
                nc.vector.tensor_copy(
                    out=vT_sb[:, rc, mc * P : (mc + 1) * P], in_=pt[:]
                )

    for bt in range(NBT):
        b0 = bt * BT
        dataT = dTpool.tile([P, KC, BT], F32R)
        ctxT = cTpool.tile([P, CC, BT], F32R)
        for sb in range(BT // P):
            row = b0 + sb * P
            dn = dpool.tile([P, N_IN], F32R)
            nc.sync.dma_start(out=dn[:], in_=d_data.ap()[row : row + P, :])
            for kc in range(KC):
                pt = tp_psum.tile([P, P], F32R)
                nc.tensor.transpose(
                    pt[:], dn[:, kc * P : (kc + 1) * P], ident_r
                )
                nc.vector.tensor_copy(
                    out=dataT[:, kc, sb * P : (sb + 1) * P], in_=pt[:]
                )
            cn = cpool.tile([P, CCTX], F32R)
            nc.sync.dma_start(out=cn[:], in_=d_ctx.ap()[row : row + P, :])
            for cc in range(CC):
                pt = tp_psum.tile([P, P], F32R)
                nc.tensor.transpose(
                    pt[:], cn[:, cc * P : (cc + 1) * P], ident_r
                )
                nc.vector.tensor_copy(
                    out=ctxT[:, cc, sb * P : (sb + 1) * P], in_=pt[:]
                )

        # lowT[r, b] = (u.T @ data.T) * (s + w.T @ ctx.T), per 128-rank chunk
        lowT = lowpool.tile([P, RC, BT], F32R)
        for rc in range(RC):
            ps = s_psum.tile([P, BT], F32)
            for cc in range(CC):
                nc.tensor.matmul(
                    ps[:],
                    lhsT=w_sb[:, cc, rc * P : (rc + 1) * P],
                    rhs=ctxT[:, cc, :],
                    start=(cc == 0),
                    stop=(cc == CC - 1),
                )
            smod = smodpool.tile([P, BT], F32)
            nc.scalar.add(smod[:], ps[:], add=s_sb[:, rc : rc + 1])
            pd = du_psum.tile([P, BT], F32)
            for kc in range(KC):
                nc.tensor.matmul(
                    pd[:],
                    lhsT=u_sb[:, kc, rc * P : (rc + 1) * P],
                    rhs=dataT[:, kc, :],
                    start=(kc == 0),
                    stop=(kc == KC - 1),
                )
            nc.vector.tensor_mul(out=lowT[:, rc, :], in0=pd[:], in1=smod[:])

        # out[b, m] = relu(low @ v.T + 2*bias) over 128-row x 512-col tiles
        for ms in range(NMS):
            for bc in range(BT // P):
                po = o_psum.tile([P, MS], F32)
                for rc in range(RC):
                    nc.tensor.matmul(
                        po[:],
                        lhsT=lowT[:, rc, bc * P : (bc + 1) * P],
                        rhs=vT_sb[:, rc, ms * MS : (ms + 1) * MS],
                        start=(rc == 0),
                        stop=False,
                    )
                nc.tensor.matmul(
                    po[:],
                    lhsT=ones[:, :],
                    rhs=bias2[:, ms * MS : (ms + 1) * MS],
                    start=False,
                    stop=True,
                )
                osb = opool.tile([P, MS], F32)
                nc.scalar.activation(osb[:], po[:], mybir.ActivationFunctionType.Relu)
                nc.sync.dma_start(
                    out=d_out.ap()[
                        b0 + bc * P : b0 + (bc + 1) * P, ms * MS : (ms + 1) * MS
                    ],
                    in_=osb[:],
                )


_CACHE = {}


def build():
    if "nc" in _CACHE:
        return _CACHE["nc"]
    nc = bacc.Bacc("TRN2", target_bir_lowering=False, debug=False)
    with tile.TileContext(nc) as tc, ExitStack() as ctx:
        _emit(nc, tc, ctx)
    nc.compile()
    _CACHE["nc"] = nc
    return nc


def kernel(data, context, u, s, v, w, bias):
    nc = build()
    in_maps = []
    for c in range(NCORES):
        sl = slice(c * NB, (c + 1) * NB)
        in_maps.append(
            {
                "data": np.ascontiguousarray(data[sl], dtype=np.float32),
                "context": np.ascontiguousarray(context[sl], dtype=np.float32),
                "u": np.asarray(u, dtype=np.float32),
                "s": np.asarray(s, dtype=np.float32),
                "v": np.asarray(v, dtype=np.float32),
                "w": np.asarray(w, dtype=np.float32),
                "bias": np.asarray(bias, dtype=np.float32),
            }
        )
    res = run_bass_kernel_spmd(nc, in_maps, core_ids=list(range(NCORES)))
    return np.concatenate([r["out"] for r in res.results], axis=0)


# revision 6
# speedup vs baseline: 1.3702x; 1.3702x over previous
"""Trainium2 Bass kernel for nn_CADense (context-adaptive low-rank dense layer).

Computes, for the full batch:
    s_mod = s + context @ w          # [B, R]
    low   = (data @ u) * s_mod       # [B, R]
    out   = relu(low @ v.T + 2*bias) # [B, UNITS]

Sharding: data-parallel over batch across 8 NeuronCores; u/s/v/w/bias
replicated. Each core runs the same Bass program on its 1024-row shard.

The PE contracts over the partition dim, so the big operands are marshaled
host-side into contraction-major layouts (data.T, context.T, v.T) when the
shards are built — on-chip PE transposes would otherwise dominate the
kernel. All matmuls run as float32r (full-rate fp32 streaming mode).

Compute is done in the "transposed" domain per rank-chunk:
    lowT[r, b] = (u.T @ data.T)[r, b] * (s[r] + (w.T @ ctx.T)[r, b])
with the s-add fused into the scalar-engine PSUM evacuation, then the
final matmul returns to natural [b, units] layout with the 2*bias add
folded in as a K=1 rank-1 matmul and ReLU fused into PSUM evacuation.
"""

import os
import sys
from contextlib import ExitStack

import numpy as np


def _ensure_concourse():
    try:
        import concourse  # noqa: F401
    except ImportError:
        for p in ("/opt/trn_rl_repo", "/root/.axon_site/_ro/trn_rl_repo"):
            if os.path.isdir(p) and p not in sys.path:
                sys.path.insert(0, p)


_ensure_concourse()

import concourse.tile as tile  # noqa: E402
from concourse import bacc, mybir  # noqa: E402
from concourse.bass_utils import run_bass_kernel_spmd  # noqa: E402

NCORES = 8
B, N_IN, UNITS, RANK, CCTX = 8192, 2048, 2048, 256, 512
NB = B // NCORES  # batch rows per core
P = 128
BT = 512  # batch tile (free dim of T-domain matmuls)
NBT = NB // BT  # batch tiles per core
KC = N_IN // P  # 16 contraction chunks for data @ u
CC = CCTX // P  # 4 contraction chunks for context @ w
RC = RANK // P  # 2 rank chunks
MS = 512  # output units slice width
NMS = UNITS // MS  # 4 unit slices

F32 = mybir.dt.float32
F32R = mybir.dt.float32r


def _emit(nc, tc, ctx):
    # Host-marshaled transposed layouts: dataT = data.T, ctxT = context.T,
    # vT = v.T (built per-shard in kernel()).
    d_dataT = nc.dram_tensor("dataT", [N_IN, NB], F32R, kind="ExternalInput")
    d_ctxT = nc.dram_tensor("ctxT", [CCTX, NB], F32R, kind="ExternalInput")
    d_u = nc.dram_tensor("u", [N_IN, RANK], F32R, kind="ExternalInput")
    d_s = nc.dram_tensor("s", [RANK], F32, kind="ExternalInput")
    d_vT = nc.dram_tensor("vT", [RANK, UNITS], F32R, kind="ExternalInput")
    d_w = nc.dram_tensor("w", [CCTX, RANK], F32R, kind="ExternalInput")
    d_bias = nc.dram_tensor("bias", [UNITS], F32R, kind="ExternalInput")
    d_out = nc.dram_tensor("out", [NB, UNITS], F32, kind="ExternalOutput")

    ap_dataT = d_dataT.ap().rearrange("(kc p) b -> p kc b", p=P)
    ap_ctxT = d_ctxT.ap().rearrange("(cc p) b -> p cc b", p=P)

    singles = ctx.enter_context(tc.tile_pool(name="singles", bufs=1))
    du_psum = ctx.enter_context(tc.tile_pool(name="du_psum", bufs=2, space="PSUM"))
    s_psum = ctx.enter_context(tc.tile_pool(name="s_psum", bufs=2, space="PSUM"))
    o_psum = ctx.enter_context(tc.tile_pool(name="o_psum", bufs=4, space="PSUM"))
    dTpool = ctx.enter_context(tc.tile_pool(name="dataT", bufs=2))
    cTpool = ctx.enter_context(tc.tile_pool(name="ctxT", bufs=2))
    lowpool = ctx.enter_context(tc.tile_pool(name="lowT", bufs=2))
    smodpool = ctx.enter_context(tc.tile_pool(name="smod", bufs=2))
    opool = ctx.enter_context(tc.tile_pool(name="outsb", bufs=3))

    # Small replicated operands, loaded once.
    w_sb = singles.tile([P, CC, RANK], F32R)
    nc.sync.dma_start(out=w_sb[:], in_=d_w.ap().rearrange("(cc p) r -> p cc r", p=P))
    s_sb = singles.tile([P, RC], F32)
    nc.sync.dma_start(out=s_sb[:], in_=d_s.ap().rearrange("(rc p) -> p rc", p=P))

    # u and the first batch-tile of dataT are interleaved per k-chunk so the
    # first accumulation group starts as soon as its first operands land.
    u_sb = singles.tile([P, KC, RANK], F32R)
    ap_u = d_u.ap().rearrange("(kc p) r -> p kc r", p=P)
    dataT0 = dTpool.tile([P, KC, BT], F32R, tag="dataT")
    ctxT0 = cTpool.tile([P, CC, BT], F32R, tag="ctxT")
    nc.sync.dma_start(out=ctxT0[:], in_=ap_ctxT[:, :, 0:BT])
    for kc in range(KC):
        nc.sync.dma_start(out=u_sb[:, kc], in_=ap_u[:, kc])
        nc.sync.dma_start(out=dataT0[:, kc], in_=ap_dataT[:, kc, 0:BT])

    bias2 = singles.tile([1, UNITS], F32R)
    nc.sync.dma_start(out=bias2[:], in_=d_bias.ap().rearrange("(a m) -> a m", a=1))
    ones_f = singles.tile([1, P], F32)
    nc.vector.memset(ones_f[:], 2.0)
    ones = singles.tile([1, P], F32R)
    nc.vector.tensor_copy(out=ones[:], in_=ones_f[:])
    vT_sb = singles.tile([P, RC, UNITS], F32R)
    nc.sync.dma_start(
        out=vT_sb[:], in_=d_vT.ap().rearrange("(rc p) m -> p rc m", p=P)
    )

    for bt in range(NBT):
        b0 = bt * BT
        if bt == 0:
            dataT, ctxT = dataT0, ctxT0
        else:
            dataT = dTpool.tile([P, KC, BT], F32R, tag="dataT")
            nc.sync.dma_start(out=dataT[:], in_=ap_dataT[:, :, b0 : b0 + BT])
            ctxT = cTpool.tile([P, CC, BT], F32R, tag="ctxT")
            nc.sync.dma_start(out=ctxT[:], in_=ap_ctxT[:, :, b0 : b0 + BT])

        # lowT[r, b] = (u.T @ data.T) * (s + w.T @ ctx.T), per 128-rank chunk
        lowT = lowpool.tile([P, RC, BT], F32R)
        for rc in range(RC):
            pd = du_psum.tile([P, BT], F32)
            for kc in range(KC):
                nc.tensor.matmul(
                    pd[:],
                    lhsT=u_sb[:, kc, rc * P : (rc + 1) * P],
                    rhs=dataT[:, kc, :],
                    start=(kc == 0),
                    stop=(kc == KC - 1),
                )
            ps = s_psum.tile([P, BT], F32)
            for cc in range(CC):
                nc.tensor.matmul(
                    ps[:],
                    lhsT=w_sb[:, cc, rc * P : (rc + 1) * P],
                    rhs=ctxT[:, cc, :],
                    start=(cc == 0),
                    stop=(cc == CC - 1),
                )
            smod = smodpool.tile([P, BT], F32)
            nc.scalar.add(smod[:], ps[:], add=s_sb[:, rc : rc + 1])
            nc.vector.tensor_mul(out=lowT[:, rc, :], in0=pd[:], in1=smod[:])

        # out[b, m] = relu(low @ v.T + 2*bias) over 128-row x 512-col tiles
        for bc in range(BT // P):
            osb = opool.tile([P, UNITS], F32)
            for ms in range(NMS):
                po = o_psum.tile([P, MS], F32)
                for rc in range(RC):
                    nc.tensor.matmul(
                        po[:],
                        lhsT=lowT[:, rc, bc * P : (bc + 1) * P],
                        rhs=vT_sb[:, rc, ms * MS : (ms + 1) * MS],
                        start=(rc == 0),
                        stop=False,
                    )
                nc.tensor.matmul(
                    po[:],
                    lhsT=ones[:, :],
                    rhs=bias2[:, ms * MS : (ms + 1) * MS],
                    start=False,
                    stop=True,
                )
                nc.scalar.activation(
                    osb[:, ms * MS : (ms + 1) * MS],
                    po[:],
                    mybir.ActivationFunctionType.Relu,
                )
            nc.sync.dma_start(
                out=d_out.ap()[b0 + bc * P : b0 + (bc + 1) * P, :], in_=osb[:]
            )


_CACHE = {}


def build():
    if "nc" in _CACHE:
        return _CACHE["nc"]
    nc = bacc.Bacc("TRN2", target_bir_lowering=False, debug=False)
    with tile.TileContext(nc) as tc, ExitStack() as ctx:
        _emit(nc, tc, ctx)
    nc.compile()
    _CACHE["nc"] = nc
    return nc


def make_in_maps(data, context, u, s, v, w, bias):
    u = np.ascontiguousarray(np.asarray(u, dtype=np.float32))
    s = np.ascontiguousarray(np.asarray(s, dtype=np.float32))
    vT = np.ascontiguousarray(np.asarray(v, dtype=np.float32).T)
    w = np.ascontiguousarray(np.asarray(w, dtype=np.float32))
    bias = np.ascontiguousarray(np.asarray(bias, dtype=np.float32))
    in_maps = []
    for c in range(NCORES):
        sl = slice(c * NB, (c + 1) * NB)
        in_maps.append(
            {
                "dataT": np.ascontiguousarray(np.asarray(data[sl], dtype=np.float32).T),
                "ctxT": np.ascontiguousarray(
                    np.asarray(context[sl], dtype=np.float32).T
                ),
                "u": u,
                "s": s,
                "vT": vT,
                "w": w,
                "bias": bias,
            }
        )
    return in_maps


def kernel(data, context, u, s, v, w, bias):
    nc = build()
    in_maps = make_in_maps(data, context, u, s, v, w, bias)
    res = run_bass_kernel_spmd(nc, in_maps, core_ids=list(range(NCORES)))
    return np.concatenate([r["out"] for r in res.results], axis=0)


# revision 9
# speedup vs baseline: 1.6926x; 1.2352x over previous
"""Trainium2 Bass kernel for nn_CADense (context-adaptive low-rank dense layer).

Computes, for the full batch:
    s_mod = s + context @ w          # [B, R]
    low   = (data @ u) * s_mod       # [B, R]
    out   = relu(low @ v.T + 2*bias) # [B, UNITS]

Sharding: data-parallel over batch across 8 NeuronCores; u/s/v/w/bias
replicated. Each core runs the same Bass program on its 1024-row shard.

The PE contracts over the partition dim, so the big operands are marshaled
host-side into contraction-major layouts (data.T, context.T, v.T) when the
shards are built — on-chip PE transposes would otherwise dominate the
kernel. All matmuls run as float32r (full-rate fp32 streaming mode).

Compute is done in the "transposed" domain per rank-chunk:
    lowT[r, b] = (u.T @ data.T)[r, b] * (s[r] + (w.T @ ctx.T)[r, b])
with the s-add fused into the scalar-engine PSUM evacuation. The final
matmul returns to natural [b, units] layout; the 2*bias add runs on the
(otherwise idle) vector engine and ReLU on the scalar engine.

The two 512-row batch tiles are software-pipelined: the PE emission
interleaves batch-tile 1's rank-stage matmuls with batch-tile 0's output
stage so the PE never idles long enough for the HAM clock gate to
re-throttle, and all input DMAs are queued up front with per-k-chunk
tiles so the first accumulation group starts as soon as its first
operands land.
"""

import os
import sys
from contextlib import ExitStack

import numpy as np


def _ensure_concourse():
    try:
        import concourse  # noqa: F401
    except ImportError:
        for p in ("/opt/trn_rl_repo", "/root/.axon_site/_ro/trn_rl_repo"):
            if os.path.isdir(p) and p not in sys.path:
                sys.path.insert(0, p)


_ensure_concourse()

import concourse.bass as bass  # noqa: E402
import concourse.tile as tile  # noqa: E402
from concourse import bacc, mybir  # noqa: E402
from concourse.bass_utils import run_bass_kernel_spmd  # noqa: E402

NCORES = 8
B, N_IN, UNITS, RANK, CCTX = 8192, 2048, 2048, 256, 512
NB = B // NCORES  # batch rows per core
P = 128
BT = 512  # batch tile (free dim of T-domain matmuls)
NBT = NB // BT  # batch tiles per core
KC = N_IN // P  # 16 contraction chunks for data @ u
CC = CCTX // P  # 4 contraction chunks for context @ w
RC = RANK // P  # 2 rank chunks
MS = 512  # output units slice width
NMS = UNITS // MS  # 4 unit slices

F32 = mybir.dt.float32
F32R = mybir.dt.float32r


def _emit(nc, tc, ctx):
    # Host-marshaled transposed layouts: dataT = data.T, ctxT = context.T,
    # vT = v.T (built per-shard in kernel()).
    d_dataT = nc.dram_tensor("dataT", [N_IN, NB], F32R, kind="ExternalInput")
    d_ctxT = nc.dram_tensor("ctxT", [CCTX, NB], F32R, kind="ExternalInput")
    d_u = nc.dram_tensor("u", [N_IN, RANK], F32R, kind="ExternalInput")
    d_s = nc.dram_tensor("s", [RANK], F32, kind="ExternalInput")
    d_vT = nc.dram_tensor("vT", [RANK, UNITS], F32R, kind="ExternalInput")
    d_w = nc.dram_tensor("w", [CCTX, RANK], F32R, kind="ExternalInput")
    d_bias = nc.dram_tensor("bias", [UNITS], F32, kind="ExternalInput")
    d_out = nc.dram_tensor("out", [NB, UNITS], F32, kind="ExternalOutput")

    ap_dataT = d_dataT.ap().rearrange("(kc p) b -> p kc b", p=P)
    ap_ctxT = d_ctxT.ap().rearrange("(cc p) b -> p cc b", p=P)
    ap_u = d_u.ap().rearrange("(kc p) r -> p kc r", p=P)
    ap_vT = d_vT.ap().rearrange("(rc p) m -> p rc m", p=P)

    singles = ctx.enter_context(tc.tile_pool(name="singles", bufs=1))
    du_psum = ctx.enter_context(tc.tile_pool(name="du_psum", bufs=2, space="PSUM"))
    s_psum = ctx.enter_context(tc.tile_pool(name="s_psum", bufs=2, space="PSUM"))
    o_psum = ctx.enter_context(tc.tile_pool(name="o_psum", bufs=4, space="PSUM"))
    dTpool = ctx.enter_context(tc.tile_pool(name="dataT", bufs=1))
    cTpool = ctx.enter_context(tc.tile_pool(name="ctxT", bufs=2))
    lowpool = ctx.enter_context(tc.tile_pool(name="lowT", bufs=2))
    smodpool = ctx.enter_context(tc.tile_pool(name="smod", bufs=2))
    opool = ctx.enter_context(tc.tile_pool(name="outsb", bufs=2))

    # ---- input DMA queue, in consumption order -------------------------
    w_sb = singles.tile([P, CC, RANK], F32R)
    nc.sync.dma_start(out=w_sb[:], in_=d_w.ap().rearrange("(cc p) r -> p cc r", p=P))
    s_sb = singles.tile([P, RC], F32)
    nc.sync.dma_start(out=s_sb[:], in_=d_s.ap().rearrange("(rc p) -> p rc", p=P))
    ctxT_t = {}
    for bt in range(NBT):
        ctxT_t[bt] = cTpool.tile([P, CC, BT], F32R, tag="ctxT", name="ctxT")
        nc.sync.dma_start(
            out=ctxT_t[bt][:], in_=ap_ctxT[:, :, bt * BT : (bt + 1) * BT]
        )

    # u and the first batch-tile of dataT interleaved per k-chunk so the
    # first accumulation group starts as soon as its first operands land.
    u_t = []
    dataT_t = {}
    dataT_t[0] = []
    for kc in range(KC):
        ut = singles.tile([P, RANK], F32R, tag=f"u{kc}", name=f"u{kc}")
        nc.sync.dma_start(out=ut[:], in_=ap_u[:, kc])
        u_t.append(ut)
        dt = dTpool.tile([P, BT], F32R, tag=f"dataT{kc}", name=f"dataT{kc}")
        nc.sync.dma_start(out=dt[:], in_=ap_dataT[:, kc, 0:BT])
        dataT_t[0].append(dt)

    vT_sb = singles.tile([P, RC, UNITS], F32R)
    nc.sync.dma_start(out=vT_sb[:, 0], in_=ap_vT[:, 0])
    bias_bc = singles.tile([P, UNITS], F32)
    nc.gpsimd.dma_start(
        out=bias_bc[:],
        in_=bass.AP(
            tensor=d_bias.ap().tensor, offset=0, ap=[[0, P], [1, UNITS]]
        ),
    )
    nc.scalar.mul(bias_bc[:], bias_bc[:], 2.0)
    nc.sync.dma_start(out=vT_sb[:, 1], in_=ap_vT[:, 1])

    for bt in range(1, NBT):
        dataT_t[bt] = []
        for kc4 in range(KC // 4):
            big = dTpool.tile([P, 4, BT], F32R, tag=f"dataTq{kc4}", name=f"dataTq{kc4}")
            nc.sync.dma_start(
                out=big[:], in_=ap_dataT[:, kc4 * 4 : (kc4 + 1) * 4, bt * BT :]
            )
            for j in range(4):
                dataT_t[bt].append(big[:, j])

    # ---- compute stages ------------------------------------------------
    lowT_t = {}

    def emit_rank_stage(bt, rc, half):
        """mm1T k-chunks for one half; second half adds mm2T + smod + mul."""
        if half == 0:
            pd = du_psum.tile([P, BT], F32, tag="pd", name="pd")
            emit_rank_stage.pd[(bt, rc)] = pd
            for kc in range(KC // 2):
                dt = dataT_t[bt][kc]
                nc.tensor.matmul(
                    pd[:],
                    lhsT=u_t[kc][:, rc * P : (rc + 1) * P],
                    rhs=dt if isinstance(dt, bass.AP) else dt[:],
                    start=(kc == 0),
                    stop=False,
                )
        else:
            pd = emit_rank_stage.pd[(bt, rc)]
            for kc in range(KC // 2, KC):
                dt = dataT_t[bt][kc]
                nc.tensor.matmul(
                    pd[:],
                    lhsT=u_t[kc][:, rc * P : (rc + 1) * P],
                    rhs=dt if isinstance(dt, bass.AP) else dt[:],
                    start=False,
                    stop=(kc == KC - 1),
                )
            ps = s_psum.tile([P, BT], F32, tag="ps", name="ps")
            for cc in range(CC):
                nc.tensor.matmul(
                    ps[:],
                    lhsT=w_sb[:, cc, rc * P : (rc + 1) * P],
                    rhs=ctxT_t[bt][:, cc, :],
                    start=(cc == 0),
                    stop=(cc == CC - 1),
                )
            smod = smodpool.tile([P, BT], F32, tag="smod", name="smod")
            nc.scalar.add(smod[:], ps[:], add=s_sb[:, rc : rc + 1])
            if bt not in lowT_t:
                lowT_t[bt] = lowpool.tile([P, RC, BT], F32R, tag="lowT", name="lowT")
            nc.vector.tensor_mul(out=lowT_t[bt][:, rc, :], in0=pd[:], in1=smod[:])

    emit_rank_stage.pd = {}

    def emit_out_stage(bt, bc):
        """out[b, :] = relu(low @ v.T + 2*bias) for one 128-row chunk."""
        b0 = bt * BT
        lowT = lowT_t[bt]
        osb = opool.tile([P, UNITS], F32, tag="osb", name="osb")
        for ms in range(NMS):
            po = o_psum.tile([P, MS], F32, tag="po", name="po")
            for rc in range(RC):
                nc.tensor.matmul(
                    po[:],
                    lhsT=lowT[:, rc, bc * P : (bc + 1) * P],
                    rhs=vT_sb[:, rc, ms * MS : (ms + 1) * MS],
                    start=(rc == 0),
                    stop=(rc == RC - 1),
                )
            sl = slice(ms * MS, (ms + 1) * MS)
            nc.vector.tensor_add(out=osb[:, sl], in0=po[:], in1=bias_bc[:, sl])
            nc.scalar.activation(
                osb[:, sl], osb[:, sl], mybir.ActivationFunctionType.Relu
            )
        nc.sync.dma_start(
            out=d_out.ap()[b0 + bc * P : b0 + (bc + 1) * P, :], in_=osb[:]
        )

    # Software pipeline: bt0 rank stage, then interleave bt0 output stage
    # with bt1 rank stage, then bt1 output stage.
    emit_rank_stage(0, 0, 0)
    emit_rank_stage(0, 0, 1)
    emit_rank_stage(0, 1, 0)
    emit_rank_stage(0, 1, 1)
    emit_out_stage(0, 0)
    emit_rank_stage(1, 0, 0)
    emit_out_stage(0, 1)
    emit_rank_stage(1, 0, 1)
    emit_out_stage(0, 2)
    emit_rank_stage(1, 1, 0)
    emit_out_stage(0, 3)
    emit_rank_stage(1, 1, 1)
    for bc in range(BT // P):
        emit_out_stage(1, bc)


_CACHE = {}


def build():
    if "nc" in _CACHE:
        return _CACHE["nc"]
    nc = bacc.Bacc("TRN2", target_bir_lowering=False, debug=False)
    with tile.TileContext(nc) as tc, ExitStack() as ctx:
        _emit(nc, tc, ctx)
    nc.compile()
    _CACHE["nc"] = nc
    return nc


def make_in_maps(data, context, u, s, v, w, bias):
    u = np.ascontiguousarray(np.asarray(u, dtype=np.float32))
    s = np.ascontiguousarray(np.asarray(s, dtype=np.float32))
    vT = np.ascontiguousarray(np.asarray(v, dtype=np.float32).T)
    w = np.ascontiguousarray(np.asarray(w, dtype=np.float32))
    bias = np.ascontiguousarray(np.asarray(bias, dtype=np.float32))
    in_maps = []
    for c in range(NCORES):
        sl = slice(c * NB, (c + 1) * NB)
        in_maps.append(
            {
                "dataT": np.ascontiguousarray(np.asarray(data[sl], dtype=np.float32).T),
                "ctxT": np.ascontiguousarray(
                    np.asarray(context[sl], dtype=np.float32).T
                ),
                "u": u,
                "s": s,
                "vT": vT,
                "w": w,
                "bias": bias,
            }
        )
    return in_maps


def kernel(data, context, u, s, v, w, bias):
    nc = build()
    in_maps = make_in_maps(data, context, u, s, v, w, bias)
    res = run_bass_kernel_spmd(nc, in_maps, core_ids=list(range(NCORES)))
    return np.concatenate([r["out"] for r in res.results], axis=0)


# revision 10
# speedup vs baseline: 1.7148x; 1.0131x over previous
"""Trainium2 Bass kernel for nn_CADense (context-adaptive low-rank dense layer).

Computes, for the full batch:
    s_mod = s + context @ w          # [B, R]
    low   = (data @ u) * s_mod       # [B, R]
    out   = relu(low @ v.T + 2*bias) # [B, UNITS]

Sharding: data-parallel over batch across 8 NeuronCores; u/s/v/w/bias
replicated. Each core runs the same Bass program on its 1024-row shard.

The PE contracts over the partition dim, so the big operands are marshaled
host-side into contraction-major layouts (data.T, context.T, v.T) when the
shards are built — on-chip PE transposes would otherwise dominate the
kernel. All matmuls run as float32r (full-rate fp32 streaming mode).

Compute is done in the "transposed" domain per rank-chunk:
    lowT[r, b] = (u.T @ data.T)[r, b] * (s[r] + (w.T @ ctx.T)[r, b])
with the s-add fused into the scalar-engine PSUM evacuation. The final
matmul returns to natural [b, units] layout; the 2*bias add is folded in
as a K=1 rank-1 matmul into the same PSUM accumulation group and ReLU is
fused into the scalar-engine PSUM evacuation.

Schedule notes:
- Input DMAs are queued up front in first-use order, sized ~0.25-1 MiB,
  with per-chunk tiles so each accumulation step fires as its operands
  land.
- The two 512-row batch tiles are software-pipelined; PE emission
  interleaves batch-tile 1's rank stage with batch-tile 0's output stage
  so the PE never idles long enough for the HAM clock gate to re-throttle.
- A short burst of bf16 dummy matmuls on garbage SBUF pre-warms the HAM
  clock gate while the first DMAs stream in.
- Output stores issue from the scalar-engine HWDGE queue so they do not
  serialize against loads on the sync-engine queue.
"""

import os
import sys
from contextlib import ExitStack

import numpy as np


def _ensure_concourse():
    try:
        import concourse  # noqa: F401
    except ImportError:
        for p in ("/opt/trn_rl_repo", "/root/.axon_site/_ro/trn_rl_repo"):
            if os.path.isdir(p) and p not in sys.path:
                sys.path.insert(0, p)


_ensure_concourse()

import concourse.tile as tile  # noqa: E402
from concourse import bacc, mybir  # noqa: E402
from concourse.bass_utils import run_bass_kernel_spmd  # noqa: E402

NCORES = 8
B, N_IN, UNITS, RANK, CCTX = 8192, 2048, 2048, 256, 512
NB = B // NCORES  # batch rows per core
P = 128
BT = 512  # batch tile (free dim of T-domain matmuls)
NBT = NB // BT  # batch tiles per core
KC = N_IN // P  # 16 contraction chunks for data @ u
CC = CCTX // P  # 4 contraction chunks for context @ w
RC = RANK // P  # 2 rank chunks
MS = 512  # output units slice width
NMS = UNITS // MS  # 4 unit slices
N_WARMUP_MM = 14

F32 = mybir.dt.float32
F32R = mybir.dt.float32r
BF16 = mybir.dt.bfloat16


def _emit(nc, tc, ctx):
    # Host-marshaled transposed layouts: dataT = data.T, ctxT = context.T,
    # vT = v.T (built per-shard in kernel()).
    d_dataT = nc.dram_tensor("dataT", [N_IN, NB], F32R, kind="ExternalInput")
    d_ctxT = nc.dram_tensor("ctxT", [CCTX, NB], F32R, kind="ExternalInput")
    d_u = nc.dram_tensor("u", [N_IN, RANK], F32R, kind="ExternalInput")
    d_s = nc.dram_tensor("s", [RANK], F32, kind="ExternalInput")
    d_vT = nc.dram_tensor("vT", [RANK, UNITS], F32R, kind="ExternalInput")
    d_w = nc.dram_tensor("w", [CCTX, RANK], F32R, kind="ExternalInput")
    d_bias = nc.dram_tensor("bias", [UNITS], F32R, kind="ExternalInput")
    d_out = nc.dram_tensor("out", [NB, UNITS], F32, kind="ExternalOutput")

    ap_dataT = d_dataT.ap().rearrange("(kc p) b -> p kc b", p=P)
    ap_ctxT = d_ctxT.ap().rearrange("(cc p) b -> p cc b", p=P)
    ap_u = d_u.ap().rearrange("(uq j p) r -> p uq j r", p=P, j=4)
    ap_vT = d_vT.ap().rearrange("(rc p) m -> p rc m", p=P)

    singles = ctx.enter_context(tc.tile_pool(name="singles", bufs=1))
    du_psum = ctx.enter_context(tc.tile_pool(name="du_psum", bufs=2, space="PSUM"))
    s_psum = ctx.enter_context(tc.tile_pool(name="s_psum", bufs=2, space="PSUM"))
    o_psum = ctx.enter_context(tc.tile_pool(name="o_psum", bufs=4, space="PSUM"))
    dTpool = ctx.enter_context(tc.tile_pool(name="dataT", bufs=1))
    cTpool = ctx.enter_context(tc.tile_pool(name="ctxT", bufs=2))
    lowpool = ctx.enter_context(tc.tile_pool(name="lowT", bufs=2))
    smodpool = ctx.enter_context(tc.tile_pool(name="smod", bufs=2))
    opool = ctx.enter_context(tc.tile_pool(name="outsb", bufs=3))

    # HAM warm-up fodder: garbage bf16 matmuls while the first loads stream.
    wu_a = singles.tile([P, P], BF16)
    nc.vector.memset(wu_a[:], 1.0)
    wu_b = singles.tile([P, MS], BF16)
    nc.vector.memset(wu_b[:], 1.0)

    # ---- input DMA queue (sync engine), in first-use order -------------
    u_t = []  # u_t[uq] = [P, 4, RANK] tile; chunk kc = u_t[kc//4][:, kc%4]
    dataT_t = {0: [], 1: []}

    uq0 = singles.tile([P, 4, RANK], F32R, name="uq0")
    nc.sync.dma_start(out=uq0[:], in_=ap_u[:, 0])
    u_t.append(uq0)
    for kc in range(4):
        dt = dTpool.tile([P, BT], F32R, tag=f"dataT{kc}", name=f"dataT{kc}")
        nc.sync.dma_start(out=dt[:], in_=ap_dataT[:, kc, 0:BT])
        dataT_t[0].append(dt[:])
    uq1 = singles.tile([P, 4, RANK], F32R, name="uq1")
    nc.sync.dma_start(out=uq1[:], in_=ap_u[:, 1])
    u_t.append(uq1)
    q = dTpool.tile([P, 4, BT], F32R, tag="dataT0q1", name="dataT0q1")
    nc.sync.dma_start(out=q[:], in_=ap_dataT[:, 4:8, 0:BT])
    dataT_t[0] += [q[:, j] for j in range(4)]

    w_sb = singles.tile([P, CC, RANK], F32R)
    nc.sync.dma_start(out=w_sb[:], in_=d_w.ap().rearrange("(cc p) r -> p cc r", p=P))
    ctxT_t = {}
    ctxT_t[0] = cTpool.tile([P, CC, BT], F32R, tag="ctxT", name="ctxT0")
    nc.sync.dma_start(out=ctxT_t[0][:], in_=ap_ctxT[:, :, 0:BT])

    for uq in (2, 3):
        ut = singles.tile([P, 4, RANK], F32R, name=f"uq{uq}")
        nc.sync.dma_start(out=ut[:], in_=ap_u[:, uq])
        u_t.append(ut)
        q = dTpool.tile([P, 4, BT], F32R, tag=f"dataT0q{uq}", name=f"dataT0q{uq}")
        nc.sync.dma_start(out=q[:], in_=ap_dataT[:, uq * 4 : (uq + 1) * 4, 0:BT])
        dataT_t[0] += [q[:, j] for j in range(4)]

    s_sb = singles.tile([P, RC], F32)
    nc.sync.dma_start(out=s_sb[:], in_=d_s.ap().rearrange("(rc p) -> p rc", p=P))
    bias2 = singles.tile([1, UNITS], F32R)
    nc.sync.dma_start(out=bias2[:], in_=d_bias.ap().rearrange("(a m) -> a m", a=1))
    ones_f = singles.tile([1, P], F32)
    nc.vector.memset(ones_f[:], 2.0)
    ones = singles.tile([1, P], F32R)
    nc.vector.tensor_copy(out=ones[:], in_=ones_f[:])

    vT_sb = singles.tile([P, RC, UNITS], F32R)
    nc.sync.dma_start(out=vT_sb[:, 0], in_=ap_vT[:, 0])
    nc.sync.dma_start(out=vT_sb[:, 1], in_=ap_vT[:, 1])

    for q4 in range(4):
        q = dTpool.tile([P, 4, BT], F32R, tag=f"dataT1q{q4}", name=f"dataT1q{q4}")
        nc.sync.dma_start(out=q[:], in_=ap_dataT[:, q4 * 4 : (q4 + 1) * 4, BT:])
        dataT_t[1] += [q[:, j] for j in range(4)]
        if q4 == 1:
            ctxT_t[1] = cTpool.tile([P, CC, BT], F32R, tag="ctxT", name="ctxT1")
            nc.sync.dma_start(out=ctxT_t[1][:], in_=ap_ctxT[:, :, BT:])

    # ---- HAM warm-up ---------------------------------------------------
    wu_ps = o_psum.tile([P, MS], F32, tag="po", name="wu_ps")
    for _ in range(N_WARMUP_MM):
        nc.tensor.matmul(wu_ps[:], lhsT=wu_a[:], rhs=wu_b[:], start=True, stop=True)

    # ---- compute stages ------------------------------------------------
    lowT_t = {}
    pd_t = {}

    def emit_rank_mms(bt, kc_lo, kc_hi):
        """mm1T k-chunks [kc_lo, kc_hi) for both rank chunks."""
        if kc_lo == 0:
            pd_t[bt] = [
                du_psum.tile([P, BT], F32, tag="pd", name="pd") for _ in range(RC)
            ]
        for kc in range(kc_lo, kc_hi):
            for rc in range(RC):
                nc.tensor.matmul(
                    pd_t[bt][rc][:],
                    lhsT=u_t[kc // 4][:, kc % 4, rc * P : (rc + 1) * P],
                    rhs=dataT_t[bt][kc],
                    start=(kc == 0),
                    stop=(kc == KC - 1),
                )

    def emit_rank_tail(bt):
        """mm2T + s-add + multiply, producing lowT for both rank chunks."""
        lowT_t[bt] = lowpool.tile([P, RC, BT], F32R, tag="lowT", name="lowT")
        for rc in range(RC):
            ps = s_psum.tile([P, BT], F32, tag="ps", name="ps")
            for cc in range(CC):
                nc.tensor.matmul(
                    ps[:],
                    lhsT=w_sb[:, cc, rc * P : (rc + 1) * P],
                    rhs=ctxT_t[bt][:, cc, :],
                    start=(cc == 0),
                    stop=(cc == CC - 1),
                )
            smod = smodpool.tile([P, BT], F32, tag="smod", name="smod")
            nc.scalar.add(smod[:], ps[:], add=s_sb[:, rc : rc + 1])
            nc.vector.tensor_mul(
                out=lowT_t[bt][:, rc, :], in0=pd_t[bt][rc][:], in1=smod[:]
            )

    def emit_out_stage(bt, bc):
        """out[b, :] = relu(low @ v.T + 2*bias) for one 128-row chunk."""
        b0 = bt * BT
        lowT = lowT_t[bt]
        osb = opool.tile([P, UNITS], F32, tag="osb", name="osb")
        for ms in range(NMS):
            po = o_psum.tile([P, MS], F32, tag="po", name="po")
            for rc in range(RC):
                nc.tensor.matmul(
                    po[:],
                    lhsT=lowT[:, rc, bc * P : (bc + 1) * P],
                    rhs=vT_sb[:, rc, ms * MS : (ms + 1) * MS],
                    start=(rc == 0),
                    stop=False,
                )
            nc.tensor.matmul(
                po[:],
                lhsT=ones[:],
                rhs=bias2[:, ms * MS : (ms + 1) * MS],
                start=False,
                stop=True,
            )
            nc.scalar.activation(
                osb[:, ms * MS : (ms + 1) * MS],
                po[:],
                mybir.ActivationFunctionType.Relu,
            )
        nc.scalar.dma_start(
            out=d_out.ap()[b0 + bc * P : b0 + (bc + 1) * P, :], in_=osb[:]
        )

    # Software pipeline across the two batch tiles.
    emit_rank_mms(0, 0, KC)
    emit_rank_tail(0)
    emit_out_stage(0, 0)
    emit_out_stage(0, 1)
    emit_rank_mms(1, 0, 8)
    emit_out_stage(0, 2)
    emit_rank_mms(1, 8, KC)
    emit_rank_tail(1)
    emit_out_stage(0, 3)
    for bc in range(BT // P):
        emit_out_stage(1, bc)


_CACHE = {}


def build():
    if "nc" in _CACHE:
        return _CACHE["nc"]
    nc = bacc.Bacc("TRN2", target_bir_lowering=False, debug=False)
    with tile.TileContext(nc) as tc, ExitStack() as ctx:
        _emit(nc, tc, ctx)
    nc.compile()
    _CACHE["nc"] = nc
    return nc


def make_in_maps(data, context, u, s, v, w, bias):
    u = np.ascontiguousarray(np.asarray(u, dtype=np.float32))
    s = np.ascontiguousarray(np.asarray(s, dtype=np.float32))
    vT = np.ascontiguousarray(np.asarray(v, dtype=np.float32).T)
    w = np.ascontiguousarray(np.asarray(w, dtype=np.float32))
    bias = np.ascontiguousarray(np.asarray(bias, dtype=np.float32))
    in_maps = []
    for c in range(NCORES):
        sl = slice(c * NB, (c + 1) * NB)
        in_maps.append(
            {
                "dataT": np.ascontiguousarray(np.asarray(data[sl], dtype=np.float32).T),
                "ctxT": np.ascontiguousarray(
                    np.asarray(context[sl], dtype=np.float32).T
                ),
                "u": u,
                "s": s,
                "vT": vT,
                "w": w,
                "bias": bias,
            }
        )
    return in_maps


def kernel(data, context, u, s, v, w, bias):
    nc = build()
    in_maps = make_in_maps(data, context, u, s, v, w, bias)
    res = run_bass_kernel_spmd(nc, in_maps, core_ids=list(range(NCORES)))
    return np.concatenate([r["out"] for r in res.results], axis=0)


# revision 11
# speedup vs baseline: 1.8005x; 1.0500x over previous
"""Trainium2 Bass kernel for nn_CADense (context-adaptive low-rank dense layer).

Computes, for the full batch:
    s_mod = s + context @ w          # [B, R]
    low   = (data @ u) * s_mod       # [B, R]
    out   = relu(low @ v.T + 2*bias) # [B, UNITS]

Sharding: data-parallel over batch across 8 NeuronCores; u/s/v/w/bias
replicated. Each core runs the same Bass program on its 1024-row shard.

The PE contracts over the partition dim, so the big operands are marshaled
host-side into contraction-major layouts (data.T, context.T, v.T) when the
shards are built — on-chip PE transposes would otherwise dominate the
kernel. All matmuls run as float32r (full-rate fp32 streaming mode).

Compute is done in the "transposed" domain per rank-chunk:
    lowT[r, b] = (u.T @ data.T)[r, b] * (s[r] + (w.T @ ctx.T)[r, b])
with the s-add fused into the scalar-engine PSUM evacuation. The final
matmul returns to natural [b, units] layout; the 2*bias add is folded in
as a K=1 rank-1 matmul into the same PSUM accumulation group and ReLU is
fused into the scalar-engine PSUM evacuation.

Schedule notes:
- Input DMAs are queued up front in first-use order, sized ~0.25-1 MiB,
  with per-chunk tiles so each accumulation step fires as its operands
  land.
- The two 512-row batch tiles are software-pipelined; PE emission
  interleaves batch-tile 1's rank stage with batch-tile 0's output stage
  so the PE never idles long enough for the HAM clock gate to re-throttle.
- A short burst of bf16 dummy matmuls on garbage SBUF pre-warms the HAM
  clock gate while the first DMAs stream in.
- Output stores issue from the scalar-engine HWDGE queue so they do not
  serialize against loads on the sync-engine queue.
"""

import os
import sys
from contextlib import ExitStack

import numpy as np


def _ensure_concourse():
    try:
        import concourse  # noqa: F401
    except ImportError:
        for p in ("/opt/trn_rl_repo", "/root/.axon_site/_ro/trn_rl_repo"):
            if os.path.isdir(p) and p not in sys.path:
                sys.path.insert(0, p)


_ensure_concourse()

import concourse.tile as tile  # noqa: E402
from concourse import bacc, mybir  # noqa: E402
from concourse.bass_utils import run_bass_kernel_spmd  # noqa: E402

NCORES = 8
B, N_IN, UNITS, RANK, CCTX = 8192, 2048, 2048, 256, 512
NB = B // NCORES  # batch rows per core
P = 128
BT = 512  # batch tile (free dim of T-domain matmuls)
NBT = NB // BT  # batch tiles per core
KC = N_IN // P  # 16 contraction chunks for data @ u
CC = CCTX // P  # 4 contraction chunks for context @ w
RC = RANK // P  # 2 rank chunks
MS = 512  # output units slice width
NMS = UNITS // MS  # 4 unit slices
N_WARMUP_MM = 14

F32 = mybir.dt.float32
F32R = mybir.dt.float32r
BF16 = mybir.dt.bfloat16


def _emit(nc, tc, ctx):
    # Host-marshaled transposed layouts: dataT = data.T, ctxT = context.T,
    # vT = v.T (built per-shard in kernel()).
    d_dataT = nc.dram_tensor("dataT", [N_IN, NB], F32R, kind="ExternalInput")
    d_ctxT = nc.dram_tensor("ctxT", [CCTX, NB], F32R, kind="ExternalInput")
    d_u = nc.dram_tensor("u", [N_IN, RANK], F32R, kind="ExternalInput")
    d_s = nc.dram_tensor("s", [RANK], F32, kind="ExternalInput")
    d_vT = nc.dram_tensor("vT", [RANK, UNITS], F32R, kind="ExternalInput")
    d_w = nc.dram_tensor("w", [CCTX, RANK], F32R, kind="ExternalInput")
    d_bias = nc.dram_tensor("bias", [UNITS], F32R, kind="ExternalInput")
    d_out = nc.dram_tensor("out", [NB, UNITS], F32, kind="ExternalOutput")

    ap_dataT = d_dataT.ap().rearrange("(kc p) b -> p kc b", p=P)
    ap_ctxT = d_ctxT.ap().rearrange("(cc p) b -> p cc b", p=P)
    ap_u = d_u.ap().rearrange("(uq j p) r -> p uq j r", p=P, j=4)
    ap_vT = d_vT.ap().rearrange("(rc p) m -> p rc m", p=P)

    singles = ctx.enter_context(tc.tile_pool(name="singles", bufs=1))
    du_psum = ctx.enter_context(tc.tile_pool(name="du_psum", bufs=2, space="PSUM"))
    s_psum = ctx.enter_context(tc.tile_pool(name="s_psum", bufs=1, space="PSUM"))
    o_psum = ctx.enter_context(tc.tile_pool(name="o_psum", bufs=5, space="PSUM"))
    dTpool = ctx.enter_context(tc.tile_pool(name="dataT", bufs=1))
    cTpool = ctx.enter_context(tc.tile_pool(name="ctxT", bufs=2))
    lowpool = ctx.enter_context(tc.tile_pool(name="lowT", bufs=2))
    smodpool = ctx.enter_context(tc.tile_pool(name="smod", bufs=2))
    opool = ctx.enter_context(tc.tile_pool(name="outsb", bufs=3))

    # HAM warm-up fodder: garbage bf16 matmuls while the first loads stream.
    wu_a = singles.tile([P, P], BF16)
    nc.vector.memset(wu_a[:], 1.0)
    wu_b = singles.tile([P, MS], BF16)
    nc.vector.memset(wu_b[:], 1.0)

    # ---- input DMA queue (sync engine), in first-use order -------------
    u_t = []  # u_t[uq] = [P, 4, RANK] tile; chunk kc = u_t[kc//4][:, kc%4]
    dataT_t = {0: [], 1: []}

    uq0 = singles.tile([P, 4, RANK], F32R, name="uq0")
    nc.sync.dma_start(out=uq0[:], in_=ap_u[:, 0])
    u_t.append(uq0)
    for kc in range(4):
        dt = dTpool.tile([P, BT], F32R, tag=f"dataT{kc}", name=f"dataT{kc}")
        nc.sync.dma_start(out=dt[:], in_=ap_dataT[:, kc, 0:BT])
        dataT_t[0].append(dt[:])
    uq1 = singles.tile([P, 4, RANK], F32R, name="uq1")
    nc.sync.dma_start(out=uq1[:], in_=ap_u[:, 1])
    u_t.append(uq1)
    q = dTpool.tile([P, 4, BT], F32R, tag="dataT0q1", name="dataT0q1")
    nc.sync.dma_start(out=q[:], in_=ap_dataT[:, 4:8, 0:BT])
    dataT_t[0] += [q[:, j] for j in range(4)]

    w_sb = singles.tile([P, CC, RANK], F32R)
    nc.sync.dma_start(out=w_sb[:], in_=d_w.ap().rearrange("(cc p) r -> p cc r", p=P))
    ctxT_t = {}
    ctxT_t[0] = cTpool.tile([P, CC, BT], F32R, tag="ctxT", name="ctxT0")
    nc.sync.dma_start(out=ctxT_t[0][:], in_=ap_ctxT[:, :, 0:BT])

    for uq in (2, 3):
        ut = singles.tile([P, 4, RANK], F32R, name=f"uq{uq}")
        nc.sync.dma_start(out=ut[:], in_=ap_u[:, uq])
        u_t.append(ut)
        q = dTpool.tile([P, 4, BT], F32R, tag=f"dataT0q{uq}", name=f"dataT0q{uq}")
        nc.sync.dma_start(out=q[:], in_=ap_dataT[:, uq * 4 : (uq + 1) * 4, 0:BT])
        dataT_t[0] += [q[:, j] for j in range(4)]

    s_sb = singles.tile([P, RC], F32)
    nc.sync.dma_start(out=s_sb[:], in_=d_s.ap().rearrange("(rc p) -> p rc", p=P))
    bias2 = singles.tile([1, UNITS], F32R)
    nc.sync.dma_start(out=bias2[:], in_=d_bias.ap().rearrange("(a m) -> a m", a=1))
    ones_f = singles.tile([1, P], F32)
    nc.vector.memset(ones_f[:], 2.0)
    ones = singles.tile([1, P], F32R)
    nc.vector.tensor_copy(out=ones[:], in_=ones_f[:])

    vT_sb = singles.tile([P, RC, UNITS], F32R)
    nc.sync.dma_start(out=vT_sb[:, 0], in_=ap_vT[:, 0])
    nc.sync.dma_start(out=vT_sb[:, 1], in_=ap_vT[:, 1])

    for q4 in range(4):
        q = dTpool.tile([P, 4, BT], F32R, tag=f"dataT1q{q4}", name=f"dataT1q{q4}")
        nc.sync.dma_start(out=q[:], in_=ap_dataT[:, q4 * 4 : (q4 + 1) * 4, BT:])
        dataT_t[1] += [q[:, j] for j in range(4)]
        if q4 == 1:
            ctxT_t[1] = cTpool.tile([P, CC, BT], F32R, tag="ctxT", name="ctxT1")
            nc.sync.dma_start(out=ctxT_t[1][:], in_=ap_ctxT[:, :, BT:])

    # ---- HAM warm-up ---------------------------------------------------
    wu_ps = o_psum.tile([P, MS], F32, tag="po", name="wu_ps")
    for _ in range(N_WARMUP_MM):
        nc.tensor.matmul(wu_ps[:], lhsT=wu_a[:], rhs=wu_b[:], start=True, stop=True)

    # ---- compute stages ------------------------------------------------
    lowT_t = {}
    pd_t = {}

    def emit_rank_mms(bt, kc_lo, kc_hi):
        """mm1T k-chunks [kc_lo, kc_hi) for both rank chunks."""
        if kc_lo == 0:
            pd_t[bt] = [
                du_psum.tile([P, BT], F32, tag="pd", name="pd") for _ in range(RC)
            ]
        for kc in range(kc_lo, kc_hi):
            for rc in range(RC):
                nc.tensor.matmul(
                    pd_t[bt][rc][:],
                    lhsT=u_t[kc // 4][:, kc % 4, rc * P : (rc + 1) * P],
                    rhs=dataT_t[bt][kc],
                    start=(kc == 0),
                    stop=(kc == KC - 1),
                )

    def emit_rank_tail(bt):
        """mm2T + s-add + multiply, producing lowT for both rank chunks."""
        lowT_t[bt] = lowpool.tile([P, RC, BT], F32R, tag="lowT", name="lowT")
        for rc in range(RC):
            ps = s_psum.tile([P, BT], F32, tag="ps", name="ps")
            for cc in range(CC):
                nc.tensor.matmul(
                    ps[:],
                    lhsT=w_sb[:, cc, rc * P : (rc + 1) * P],
                    rhs=ctxT_t[bt][:, cc, :],
                    start=(cc == 0),
                    stop=(cc == CC - 1),
                )
            smod = smodpool.tile([P, BT], F32, tag="smod", name="smod")
            nc.scalar.add(smod[:], ps[:], add=s_sb[:, rc : rc + 1])
            nc.vector.tensor_mul(
                out=lowT_t[bt][:, rc, :], in0=pd_t[bt][rc][:], in1=smod[:]
            )

    def emit_out_stage(bt, bc):
        """out[b, :] = relu(low @ v.T + 2*bias) for one 128-row chunk.

        All four 512-wide PSUM groups stay open at once and the matmuls
        are ordered rc-major so consecutive matmuls reuse the same
        stationary operand; ReLU evacuation alternates between the
        scalar and vector engines so neither gates PSUM recycling.
        """
        b0 = bt * BT
        lowT = lowT_t[bt]
        osb = opool.tile([P, UNITS], F32, tag="osb", name="osb")
        pos = [o_psum.tile([P, MS], F32, tag="po", name="po") for _ in range(NMS)]
        for rc in range(RC):
            for ms in range(NMS):
                nc.tensor.matmul(
                    pos[ms][:],
                    lhsT=lowT[:, rc, bc * P : (bc + 1) * P],
                    rhs=vT_sb[:, rc, ms * MS : (ms + 1) * MS],
                    start=(rc == 0),
                    stop=False,
                )
        for ms in range(NMS):
            nc.tensor.matmul(
                pos[ms][:],
                lhsT=ones[:],
                rhs=bias2[:, ms * MS : (ms + 1) * MS],
                start=False,
                stop=True,
            )
        for ms in range(NMS):
            sl = slice(ms * MS, (ms + 1) * MS)
            if ms % 2 == 0:
                nc.scalar.activation(
                    osb[:, sl], pos[ms][:], mybir.ActivationFunctionType.Relu
                )
            else:
                nc.vector.tensor_relu(out=osb[:, sl], in_=pos[ms][:])
        nc.scalar.dma_start(
            out=d_out.ap()[b0 + bc * P : b0 + (bc + 1) * P, :], in_=osb[:]
        )

    # Software pipeline across the two batch tiles.
    emit_rank_mms(0, 0, KC)
    emit_rank_tail(0)
    emit_out_stage(0, 0)
    emit_out_stage(0, 1)
    emit_rank_mms(1, 0, 8)
    emit_out_stage(0, 2)
    emit_rank_mms(1, 8, KC)
    emit_rank_tail(1)
    emit_out_stage(0, 3)
    for bc in range(BT // P):
        emit_out_stage(1, bc)


_CACHE = {}


def build():
    if "nc" in _CACHE:
        return _CACHE["nc"]
    nc = bacc.Bacc("TRN2", target_bir_lowering=False, debug=False)
    with tile.TileContext(nc) as tc, ExitStack() as ctx:
        _emit(nc, tc, ctx)
    nc.compile()
    _CACHE["nc"] = nc
    return nc


def make_in_maps(data, context, u, s, v, w, bias):
    u = np.ascontiguousarray(np.asarray(u, dtype=np.float32))
    s = np.ascontiguousarray(np.asarray(s, dtype=np.float32))
    vT = np.ascontiguousarray(np.asarray(v, dtype=np.float32).T)
    w = np.ascontiguousarray(np.asarray(w, dtype=np.float32))
    bias = np.ascontiguousarray(np.asarray(bias, dtype=np.float32))
    in_maps = []
    for c in range(NCORES):
        sl = slice(c * NB, (c + 1) * NB)
        in_maps.append(
            {
                "dataT": np.ascontiguousarray(np.asarray(data[sl], dtype=np.float32).T),
                "ctxT": np.ascontiguousarray(
                    np.asarray(context[sl], dtype=np.float32).T
                ),
                "u": u,
                "s": s,
                "vT": vT,
                "w": w,
                "bias": bias,
            }
        )
    return in_maps


def kernel(data, context, u, s, v, w, bias):
    nc = build()
    in_maps = make_in_maps(data, context, u, s, v, w, bias)
    res = run_bass_kernel_spmd(nc, in_maps, core_ids=list(range(NCORES)))
    return np.concatenate([r["out"] for r in res.results], axis=0)
